# revision 1
# baseline (speedup 1.0000x reference)
"""ConvAttention Trainium2 kernel.

Computes, for B=32 batches sharded 4-per-core across 8 NeuronCores:
  keys' = keys + style_emb^T
  k = conv1d_k1(relu(conv1d_k3(keys', kw1, kb1)), kw2, kb2)        [80, 400]
  q = conv1d_k1(relu(conv1d_k1(relu(conv1d_k3(queries, qw1, qb1)), qw2, qb2)), qw3, qb3)
  attn_raw = SCALE * (|q|^2 + |k|^2 - 2 q.k)                        [2000, 400]
  attn_logprob = log_softmax(attn_raw, axis=-1) + log(prior + EPS)
  attn = softmax(where(mask, -inf, attn_logprob), axis=-1)

Key algebraic facts used:
  * per-row (t1) constants cancel in both log_softmax and softmax, so the
    S*|q|^2 term is never computed.
  * the S*|k|^2 term is broadcast across t1 rows by a K=1 matmul with a ones
    stationary vector.
  * softmax(x + log(prior+eps), masked) == E*W / sum(E*W) with
    E = exp(x - rowmax(x)) (already needed for the logsumexp) and
    W = (prior+eps)*maskmul -- no second exponential pass is needed.
"""

import os
import sys
import numpy as np

sys.path.insert(0, "/opt/trn_rl_repo")

B, T1, T2 = 32, 2000, 400
N_MEL, N_TEXT, N_ATT = 80, 512, 80
N_CORES = 8
BPC = B // N_CORES  # batches per core
SCALE = -0.0005
EPS = 1e-8

# t1 chunking for the attention map: 15 x 128 + 80
T1_CHUNKS = [(i * 128, min(128, T1 - i * 128)) for i in range((T1 + 127) // 128)]
# t1 chunking for the q convs (psum free-dim limit 512)
QT_CHUNKS = [(i * 500, 500) for i in range(4)]
# t2 chunking for the style transpose
T2_CHUNKS = [(i * 128, min(128, T2 - i * 128)) for i in range((T2 + 127) // 128)]

_PROGRAM_CACHE = {}


def build_program(mm_f32r=True, skip_tr=False, skip_chunks=False):
    """Builds and compiles the single-core Bass program (SPMD across 8 cores)."""
    import concourse.bass as bass
    import concourse.bacc as bacc
    import concourse.mybir as mybir
    from concourse import tile

    f32 = mybir.dt.float32
    mdt = mybir.dt.float32r if mm_f32r else f32
    AF = mybir.ActivationFunctionType
    ALU = mybir.AluOpType

    nc = bacc.Bacc("TRN2", target_bir_lowering=False, debug=False,
                   num_devices=N_CORES)

    # ---- I/O -------------------------------------------------------------
    queries_h = nc.dram_tensor("queries", [BPC, N_MEL, T1], mdt, kind="ExternalInput").ap()
    keys_h = nc.dram_tensor("keys", [BPC, N_TEXT, T2], f32, kind="ExternalInput").ap()
    style_h = nc.dram_tensor("style", [BPC, T2, N_TEXT], f32, kind="ExternalInput").ap()
    prior_h = nc.dram_tensor("prior", [BPC, T1, T2], f32, kind="ExternalInput").ap()
    maskf_h = nc.dram_tensor("maskf", [BPC, T2], mdt, kind="ExternalInput").ap()
    ident_h = nc.dram_tensor("ident", [128, 128], f32, kind="ExternalInput").ap()

    qw1t_h = nc.dram_tensor("qw1t", [3, N_MEL, 2 * N_MEL], mdt, kind="ExternalInput").ap()
    qb1_h = nc.dram_tensor("qb1", [2 * N_MEL, 1], f32, kind="ExternalInput").ap()
    qw2t_h = nc.dram_tensor("qw2t", [2 * N_MEL, N_MEL], mdt, kind="ExternalInput").ap()
    qb2_h = nc.dram_tensor("qb2", [N_MEL, 1], f32, kind="ExternalInput").ap()
    qw3t_h = nc.dram_tensor("qw3t", [N_MEL, N_ATT], mdt, kind="ExternalInput").ap()
    qb3_h = nc.dram_tensor("qb3", [N_ATT, 1], f32, kind="ExternalInput").ap()
    kw1t_h = nc.dram_tensor("kw1t", [3, N_TEXT, 2 * N_TEXT], mdt, kind="ExternalInput").ap()
    kb1_h = nc.dram_tensor("kb1", [2 * N_TEXT, 1], f32, kind="ExternalInput").ap()
    # kw2ts/kb2s have -2*SCALE folded in on the host
    kw2ts_h = nc.dram_tensor("kw2ts", [2 * N_TEXT, N_ATT], mdt, kind="ExternalInput").ap()
    kb2s_h = nc.dram_tensor("kb2s", [N_ATT, 1], f32, kind="ExternalInput").ap()

    out_attn_h = nc.dram_tensor("out_attn", [BPC, T1, T2], f32, kind="ExternalOutput").ap()
    out_logp_h = nc.dram_tensor("out_logp", [BPC, T1, T2], f32, kind="ExternalOutput").ap()

    with tile.TileContext(nc) as tc:
        from contextlib import ExitStack
        with ExitStack() as ctx:
            const_pool = ctx.enter_context(tc.tile_pool(name="const", bufs=1))
            wpool = ctx.enter_context(tc.tile_pool(name="weights", bufs=1))
            kpath = ctx.enter_context(tc.tile_pool(name="kpath", bufs=2))
            y1pool = ctx.enter_context(tc.tile_pool(name="y1", bufs=10))
            qpath = ctx.enter_context(tc.tile_pool(name="qpath", bufs=2))
            chunkp = ctx.enter_context(tc.tile_pool(name="chunk", bufs=3))
            tmpp = ctx.enter_context(tc.tile_pool(name="tmp", bufs=2))
            smallp = ctx.enter_context(tc.tile_pool(name="small", bufs=4))
            ps_qk = ctx.enter_context(tc.tile_pool(name="ps_qk", bufs=3, space="PSUM"))
            ps_conv = ctx.enter_context(tc.tile_pool(name="ps_conv", bufs=3, space="PSUM"))
            ps_small = ctx.enter_context(tc.tile_pool(name="ps_small", bufs=2, space="PSUM"))

            # ---- constants & weights (loaded once) -----------------------
            ones_f1 = const_pool.tile([1, 128], f32, name="ones_f1")
            nc.vector.memset(ones_f1[:], 1.0)
            ones_f80 = const_pool.tile([N_ATT, 1], f32, name="ones_f80")
            nc.vector.memset(ones_f80[:], 1.0)
            zero_col = const_pool.tile([128, 1], f32, name="zero_col")
            nc.vector.memset(zero_col[:], 0.0)
            if mm_f32r:
                ones1 = const_pool.tile([1, 128], mdt, name="ones1")
                nc.vector.tensor_copy(ones1[:], ones_f1[:])
                ones80 = const_pool.tile([N_ATT, 1], mdt, name="ones80")
                nc.vector.tensor_copy(ones80[:], ones_f80[:])
            else:
                ones1, ones80 = ones_f1, ones_f80
            ident_sb = const_pool.tile([128, 128], f32, name="ident_sb")
            nc.sync.dma_start(out=ident_sb[:], in_=ident_h[:, :])
            eps_col = const_pool.tile([128, 1], f32, name="eps_col")
            nc.vector.memset(eps_col[:], EPS)

            kw1_sb = {}
            for d in range(3):
                for c in range(4):
                    t = wpool.tile([128, 2 * N_TEXT], mdt, name=f"kw1_{d}_{c}")
                    nc.sync.dma_start(out=t[:], in_=kw1t_h[d, 128 * c:128 * (c + 1), :])
                    kw1_sb[(d, c)] = t
            qw1_sb = []
            for d in range(3):
                t = wpool.tile([N_MEL, 2 * N_MEL], mdt, name=f"qw1_{d}")
                nc.sync.dma_start(out=t[:], in_=qw1t_h[d, :, :])
                qw1_sb.append(t)
            qw2_a = wpool.tile([128, N_MEL], mdt, name="qw2_a")
            nc.sync.dma_start(out=qw2_a[:], in_=qw2t_h[0:128, :])
            qw2_b = wpool.tile([32, N_MEL], mdt, name="qw2_b")
            nc.sync.dma_start(out=qw2_b[:], in_=qw2t_h[128:160, :])
            qw3_sb = wpool.tile([N_MEL, N_ATT], mdt, name="qw3_sb")
            nc.sync.dma_start(out=qw3_sb[:], in_=qw3t_h[:, :])
            kw2_sb = []
            for c in range(8):
                t = wpool.tile([128, N_ATT], mdt, name=f"kw2_{c}")
                nc.sync.dma_start(out=t[:], in_=kw2ts_h[128 * c:128 * (c + 1), :])
                kw2_sb.append(t)

            qb1_a = wpool.tile([128, 1], f32, name="qb1_a")
            nc.sync.dma_start(out=qb1_a[:], in_=qb1_h[0:128, :])
            qb1_b = wpool.tile([32, 1], f32, name="qb1_b")
            nc.sync.dma_start(out=qb1_b[:], in_=qb1_h[128:160, :])
            qb2_sb = wpool.tile([N_MEL, 1], f32, name="qb2_sb")
            nc.sync.dma_start(out=qb2_sb[:], in_=qb2_h[:, :])
            qb3_sb = wpool.tile([N_ATT, 1], f32, name="qb3_sb")
            nc.sync.dma_start(out=qb3_sb[:], in_=qb3_h[:, :])
            kb1_sb = []
            for c in range(8):
                t = wpool.tile([128, 1], f32, name=f"kb1_{c}")
                nc.sync.dma_start(out=t[:], in_=kb1_h[128 * c:128 * (c + 1), :])
                kb1_sb.append(t)
            kb2s_sb = wpool.tile([N_ATT, 1], f32, name="kb2s_sb")
            nc.sync.dma_start(out=kb2s_sb[:], in_=kb2s_h[:, :])

            # ---- per-batch work ------------------------------------------
            for b in range(BPC):
                # ---------- key path ----------
                st_sb = []
                for ti, (t0, tw) in enumerate(T2_CHUNKS):
                    t = kpath.tile([tw, N_TEXT], f32, name=f"st_{ti}", tag="st", bufs=6)
                    nc.sync.dma_start(out=t[:], in_=style_h[b, t0:t0 + tw, :])
                    st_sb.append(t)

                ks_sb = []  # keys+style^T, channel-major, zero-padded cols
                for c in range(4):
                    ks_ps = None
                    if not skip_tr:
                      ks_ps = ps_conv.tile([128, T2], f32, name=f"ks_ps_{c}", tag="psc")
                      for ti, (t0, tw) in enumerate(T2_CHUNKS):
                        nc.tensor.transpose(
                            ks_ps[:, t0:t0 + tw],
                            st_sb[ti][:, 128 * c:128 * (c + 1)],
                            ident_sb[0:tw, 0:tw],
                        )
                    kt = tmpp.tile([128, T2], f32, name=f"kt_{c}", tag="kt")
                    nc.sync.dma_start(out=kt[:], in_=keys_h[b, 128 * c:128 * (c + 1), :])
                    ks = kpath.tile([128, T2 + 2], mdt, name=f"ks_{c}", tag="ks", bufs=8)
                    nc.vector.tensor_copy(ks[:, 0:1], zero_col[:])
                    nc.vector.tensor_copy(ks[:, T2 + 1:T2 + 2], zero_col[:])
                    if skip_tr:
                        nc.vector.tensor_copy(ks[:, 1:T2 + 1], kt[:])
                    else:
                        nc.vector.tensor_add(ks[:, 1:T2 + 1], kt[:], ks_ps[:])
                    ks_sb.append(ks)

                # conv1 (k3, 512 -> 1024) + relu
                y1_sb = []
                for j in range(8):
                    c1 = ps_conv.tile([128, T2], f32, name=f"c1_{j}", tag="psc")
                    n = 0
                    for c in range(4):
                        for d in range(3):
                            nc.tensor.matmul(
                                c1[:],
                                kw1_sb[(d, c)][:, 128 * j:128 * (j + 1)],
                                ks_sb[c][:, d:d + T2],
                                start=(n == 0), stop=(n == 11),
                            )
                            n += 1
                    y1 = y1pool.tile([128, T2], mdt, name=f"y1_{j}", tag="y1")
                    nc.scalar.activation(y1[:], c1[:], AF.Relu, bias=kb1_sb[j][:])
                    y1_sb.append(y1)

                # conv2 (k1, 1024 -> 80), -2*SCALE folded in
                k_ps = ps_conv.tile([N_ATT, T2], f32, name="k_ps", tag="psc")
                for c in range(8):
                    nc.tensor.matmul(k_ps[:], kw2_sb[c][:], y1_sb[c][:],
                                     start=(c == 0), stop=(c == 7))
                b0 = kpath.tile([N_ATT, T2], mdt, name="b0", tag="b0")
                nc.scalar.activation(b0[:], k_ps[:], AF.Identity, bias=kb2s_sb[:])

                # S*|k|^2 row:  sum(b0^2) / (4*SCALE)
                ksq = tmpp.tile([N_ATT, T2], mdt, name="ksq", tag="ksq")
                nc.vector.tensor_mul(ksq[:], b0[:], b0[:])
                k2_ps = ps_small.tile([1, T2], f32, name="k2_ps", tag="pss")
                nc.tensor.matmul(k2_ps[:], ones80[:], ksq[:], start=True, stop=True)
                bk2 = kpath.tile([1, T2], mdt, name="bk2", tag="bk2")
                nc.scalar.activation(bk2[:], k2_ps[:], AF.Copy, scale=1.0 / (4.0 * SCALE))

                # mask row -> broadcast to [128, T2]
                mrow = smallp.tile([1, T2], mdt, name="mrow", tag="mrow")
                nc.sync.dma_start(out=mrow[0:1, :], in_=maskf_h[b:b + 1, :])
                mb_ps = ps_small.tile([128, T2], f32, name="mb_ps", tag="pss")
                nc.tensor.matmul(mb_ps[:], ones1[:], mrow[:], start=True, stop=True)
                mmul = kpath.tile([128, T2], f32, name="mmul", tag="mmul")
                nc.vector.tensor_copy(mmul[:], mb_ps[:])

                # ---------- query path ----------
                q_in = qpath.tile([N_MEL, T1 + 2], mdt, name="q_in", tag="q_in")
                nc.vector.tensor_copy(q_in[:, 0:1], zero_col[0:N_MEL, :])
                nc.vector.tensor_copy(q_in[:, T1 + 1:T1 + 2], zero_col[0:N_MEL, :])
                nc.sync.dma_start(out=q_in[:, 1:T1 + 1], in_=queries_h[b, :, :])

                q_fin = qpath.tile([N_ATT, T1], mdt, name="q_fin", tag="q_fin")
                for (t0, tw) in QT_CHUNKS:
                    y1qa = tmpp.tile([128, tw], mdt, name=f"y1qa_{t0}", tag="y1qa", bufs=3)
                    y1qb = tmpp.tile([32, tw], mdt, name=f"y1qb_{t0}", tag="y1qb", bufs=3)
                    for (p0, p1, bt, yt) in (
                        (0, 128, qb1_a, y1qa),
                        (128, 160, qb1_b, y1qb),
                    ):
                        pw = p1 - p0
                        q1 = ps_conv.tile([pw, tw], f32, name=f"q1_{t0}_{p0}", tag="psc")
                        for d in range(3):
                            nc.tensor.matmul(q1[:], qw1_sb[d][:, p0:p1],
                                             q_in[:, d + t0:d + t0 + tw],
                                             start=(d == 0), stop=(d == 2))
                        nc.scalar.activation(yt[:], q1[:], AF.Relu, bias=bt[:])

                    q2 = ps_conv.tile([N_MEL, tw], f32, name=f"q2_{t0}", tag="psc")
                    nc.tensor.matmul(q2[:], qw2_a[:], y1qa[:],
                                     start=True, stop=False)
                    nc.tensor.matmul(q2[:], qw2_b[:], y1qb[:],
                                     start=False, stop=True)
                    q_mid = tmpp.tile([N_MEL, tw], mdt, name=f"q_mid_{t0}", tag="q_mid", bufs=3)
                    nc.scalar.activation(q_mid[:], q2[:], AF.Relu, bias=qb2_sb[:])

                    q3 = ps_conv.tile([N_ATT, tw], f32, name=f"q3_{t0}", tag="psc")
                    nc.tensor.matmul(q3[:], qw3_sb[:], q_mid[:],
                                     start=True, stop=True)
                    nc.scalar.activation(q_fin[:, t0:t0 + tw], q3[:], AF.Identity, bias=qb3_sb[:])

                # ---------- attention chunks ----------
                if skip_chunks:
                    nc.sync.dma_start(out=out_attn_h[b, 0:N_ATT, :], in_=b0[:])
                    nc.sync.dma_start(out=out_logp_h[b, 0:N_ATT, :], in_=q_fin[:, 0:T2])
                    continue
                for ci, (r0, w) in enumerate(T1_CHUNKS):
                    pr = chunkp.tile([w, T2], f32, name=f"pr_{ci}", tag="pr", bufs=4)
                    nc.sync.dma_start(out=pr[:], in_=prior_h[b, r0:r0 + w, :])
                    lp = chunkp.tile([w, T2], f32, name=f"lp_{ci}", tag="lp", bufs=4)
                    nc.scalar.activation(lp[:], pr[:], AF.Ln, bias=eps_col[0:w, :])
                    # W = prior * maskmul (the +eps term is ~1e-8 relative: negligible)
                    wt_ = chunkp.tile([w, T2], f32, name=f"wt_{ci}", tag="wt", bufs=3)
                    nc.vector.tensor_mul(wt_[:], pr[:], mmul[0:w, :])

                    p0 = ps_qk.tile([w, T2], f32, name=f"p0_{ci}", tag="psqk")
                    nc.tensor.matmul(p0[:], q_fin[:, r0:r0 + w], b0[:],
                                     start=True, stop=False)
                    nc.tensor.matmul(p0[:], ones1[:, 0:w], bk2[:],
                                     start=False, stop=True)

                    # logits are bounded (|attn_raw - S*q2| <~ 1), so exp needs
                    # no max-shift; accum_out gives the row sum in the same pass
                    e = chunkp.tile([w, T2], f32, name=f"e_{ci}", tag="e", bufs=4)
                    rsum = smallp.tile([w, 1], f32, name=f"rsum_{ci}", tag="rsum")
                    nc.scalar.activation(e[:], p0[:], AF.Exp, accum_out=rsum[:])
                    lnr = smallp.tile([w, 1], f32, name=f"lnr_{ci}", tag="lnr")
                    nc.scalar.activation(lnr[:], rsum[:], AF.Ln)
                    cc = smallp.tile([w, 1], f32, name=f"cc_{ci}", tag="cc")
                    nc.vector.tensor_scalar_mul(cc[:], lnr[:], -1.0)

                    # attn_logprob = (p0 + cc) + lp
                    t1 = chunkp.tile([w, T2], f32, name=f"t1_{ci}", tag="t1", bufs=2)
                    nc.scalar.activation(t1[:], p0[:], AF.Identity, bias=cc[:])
                    o1 = chunkp.tile([w, T2], f32, name=f"o1_{ci}", tag="o1", bufs=2)
                    nc.vector.tensor_add(o1[:], t1[:], lp[:])
                    nc.sync.dma_start(out=out_logp_h[b, r0:r0 + w, :], in_=o1[:])

                    # attn = e*W / sum(e*W)
                    u = chunkp.tile([w, T2], f32, name=f"u_{ci}", tag="u", bufs=4)
                    nc.vector.tensor_mul(u[:], e[:], wt_[:])
                    rsum2 = smallp.tile([w, 1], f32, name=f"rsum2_{ci}", tag="rsum2")
                    nc.vector.tensor_reduce(rsum2[:], u[:], axis=mybir.AxisListType.X,
                                            op=ALU.add)
                    rrec = smallp.tile([w, 1], f32, name=f"rrec_{ci}", tag="rrec")
                    nc.vector.reciprocal(rrec[:], rsum2[:])
                    o2 = chunkp.tile([w, T2], f32, name=f"o2_{ci}", tag="o2", bufs=2)
                    nc.vector.tensor_scalar_mul(o2[:], u[:], rrec[:])
                    nc.sync.dma_start(out=out_attn_h[b, r0:r0 + w, :], in_=o2[:])

    nc.compile()
    return nc


def get_program(mm_f32r=True):
    key = ("prog", mm_f32r)
    if key not in _PROGRAM_CACHE:
        _PROGRAM_CACHE[key] = build_program(mm_f32r)
    return _PROGRAM_CACHE[key]


def make_in_maps(inputs):
    """Host-side prep: shard per core, transpose/fold weights."""
    queries = np.asarray(inputs["queries"], np.float32)
    keys = np.asarray(inputs["keys"], np.float32)
    mask = np.asarray(inputs["mask"])
    prior = np.asarray(inputs["attn_prior"], np.float32)
    style = np.asarray(inputs["style_emb"], np.float32)

    qw1 = np.asarray(inputs["qw1"], np.float32)
    qb1 = np.asarray(inputs["qb1"], np.float32)
    qw2 = np.asarray(inputs["qw2"], np.float32)
    qb2 = np.asarray(inputs["qb2"], np.float32)
    qw3 = np.asarray(inputs["qw3"], np.float32)
    qb3 = np.asarray(inputs["qb3"], np.float32)
    kw1 = np.asarray(inputs["kw1"], np.float32)
    kb1 = np.asarray(inputs["kb1"], np.float32)
    kw2 = np.asarray(inputs["kw2"], np.float32)
    kb2 = np.asarray(inputs["kb2"], np.float32)

    fold = -2.0 * SCALE
    qw1t = np.ascontiguousarray(qw1.transpose(2, 1, 0))        # [3, 80, 160]
    qw2t = np.ascontiguousarray(qw2[:, :, 0].T)                # [160, 80]
    qw3t = np.ascontiguousarray(qw3[:, :, 0].T)                # [80, 80]
    kw1t = np.ascontiguousarray(kw1.transpose(2, 1, 0))        # [3, 512, 1024]
    kw2ts = np.ascontiguousarray(kw2[:, :, 0].T * fold)        # [1024, 80]
    kb2s = (kb2 * fold).reshape(-1, 1).astype(np.float32)

    maskf = np.where(mask[:, :, 0], 0.0, 1.0).astype(np.float32)  # [B, 400]
    ident = np.eye(128, dtype=np.float32)

    shared = dict(
        ident=ident,
        qw1t=qw1t, qb1=qb1.reshape(-1, 1).astype(np.float32),
        qw2t=qw2t, qb2=qb2.reshape(-1, 1).astype(np.float32),
        qw3t=qw3t, qb3=qb3.reshape(-1, 1).astype(np.float32),
        kw1t=kw1t, kb1=kb1.reshape(-1, 1).astype(np.float32),
        kw2ts=kw2ts, kb2s=kb2s,
    )
    in_maps = []
    for c in range(N_CORES):
        sl = slice(c * BPC, (c + 1) * BPC)
        m = dict(shared)
        m["queries"] = np.ascontiguousarray(queries[sl])
        m["keys"] = np.ascontiguousarray(keys[sl])
        m["style"] = np.ascontiguousarray(style[sl])
        m["prior"] = np.ascontiguousarray(prior[sl])
        m["maskf"] = np.ascontiguousarray(maskf[sl])
        in_maps.append(m)
    return in_maps


def kernel(**inputs):
    from concourse.bass_utils import run_bass_kernel_spmd

    nc = get_program(os.environ.get("MM_F32R", "1") == "1")
    in_maps = make_in_maps(inputs)
    res = run_bass_kernel_spmd(nc, in_maps, list(range(N_CORES)))
    attn = np.concatenate([r["out_attn"] for r in res.results], axis=0)
    logp = np.concatenate([r["out_logp"] for r in res.results], axis=0)
    attn = attn.reshape(B, 1, T1, T2)
    logp = logp.reshape(B, 1, T1, T2)
    return attn, logp



# revision 16
# speedup vs baseline: 2.3604x; 2.3604x over previous
"""ConvAttention Trainium2 kernel (v3).

B=32 batches data-parallel, 4 per core across 8 NeuronCores.

Per batch:
  keys' = keys + style_emb^T
  k = conv1d_k1(relu(conv1d_k3(keys', kw1, kb1)), kw2, kb2)          [80, 400]
  q = conv1d_k1(relu(conv1d_k1(relu(conv1d_k3(queries)), qw2)), qw3) [80, 2000]
  x  = S*k2 + (-2S)*qk          (row-constant S*q2 dropped; S=-5e-4)  [2000, 400]
  logp = log_softmax(x) + log(prior + eps)
  attn = softmax(where(mask, -inf, logp))

Key implementation facts:
  * All wire tensors are bf16 (inputs pre-cast on host with eps folded into
    prior; outputs stored bf16 and cast back to f32 on host). Halves DMA
    bytes; tolerance is 2e-2, bf16 path lands ~1e-3.
  * The Act engine runs ONLY Exp (phase B) then ONLY Ln (phase A') per
    batch, so the activation-table loads drop from 32/batch to 2/batch.
  * logp is computed as Ln((e * prior_eps) * rrec) using the activation
    `scale` per-partition operand, where e = Exp(x) (whose accum_out gives
    the softmax denominator rsum = 1/rrec). No explicit log-prior pass.
  * attn = (u'' * maskmul) * rrec2 where u'' = e * prior_eps is shared with
    the logp path; scalar_tensor_tensor's accum_out gives the masked row
    sum in the same pass.
  * b0 and q_fin carry an 81st partition row (S*k2 row resp. ones) so the
    per-chunk logits need a single PE matmul (contract 81).
  * Conv epilogues (bias+relu) run on DVE/Pool via
    scalar_tensor_tensor(psum, bias, zeros, add, max), keeping Act clean.
  * DMAs are consolidated with AP rearrange: one strided DMA per tensor
    per batch (HWDGE costs ~630ns per DMA, serialized).
"""

import os
import sys
import numpy as np
import ml_dtypes

sys.path.insert(0, "/opt/trn_rl_repo")

B, T1, T2 = 32, 2000, 400
N_MEL, N_TEXT, N_ATT = 80, 512, 80
N_CORES = 8
BPC = B // N_CORES
SCALE = -0.0005
EPS = 1e-8
FOLD = -2.0 * SCALE          # folded into kw2/kb2 -> b0 = FOLD*k
K2SCALE = 1.0 / (4.0 * SCALE)  # bk2 = sum(b0^2) * K2SCALE = S*|k|^2

NFULL = 15                    # 15 full 128-row t1 chunks + one 80-row chunk
T1A = NFULL * 128             # 1920
W15 = T1 - T1A                # 80
QT = 500                      # q-conv t1 chunk width (psum free limit 512)

_PROGRAM_CACHE = {}


def build_program():
    import concourse.bass as bass
    import concourse.bacc as bacc
    import concourse.mybir as mybir
    from concourse import tile

    f32 = mybir.dt.float32
    bf16 = mybir.dt.bfloat16
    AF = mybir.ActivationFunctionType
    ALU = mybir.AluOpType

    nc = bacc.Bacc("TRN2", target_bir_lowering=False, debug=False,
                   num_devices=N_CORES)

    # ---- I/O (everything bf16 except f32 per-partition biases) ----------
    queries_h = nc.dram_tensor("queries", [BPC, N_MEL, T1], bf16, kind="ExternalInput").ap()
    keys_h = nc.dram_tensor("keys", [BPC, N_TEXT, T2], bf16, kind="ExternalInput").ap()
    style_h = nc.dram_tensor("style", [BPC, T2, N_TEXT], bf16, kind="ExternalInput").ap()
    prior_h = nc.dram_tensor("prior", [BPC, T1, T2], bf16, kind="ExternalInput").ap()
    maskf_h = nc.dram_tensor("maskf", [BPC, T2], bf16, kind="ExternalInput").ap()
    ident_h = nc.dram_tensor("ident", [128, 128], bf16, kind="ExternalInput").ap()

    qw1t_h = nc.dram_tensor("qw1t", [3, N_MEL, 2 * N_MEL], bf16, kind="ExternalInput").ap()
    qb1_h = nc.dram_tensor("qb1", [2 * N_MEL, 1], f32, kind="ExternalInput").ap()
    qw2t_h = nc.dram_tensor("qw2t", [2 * N_MEL, N_MEL], bf16, kind="ExternalInput").ap()
    qb2_h = nc.dram_tensor("qb2", [N_MEL, 1], f32, kind="ExternalInput").ap()
    qw3t_h = nc.dram_tensor("qw3t", [N_MEL, N_ATT], bf16, kind="ExternalInput").ap()
    qb3_h = nc.dram_tensor("qb3", [N_ATT, 1], f32, kind="ExternalInput").ap()
    kw1j_h = nc.dram_tensor("kw1j", [8, 128, 1536], bf16, kind="ExternalInput").ap()
    kb1_h = nc.dram_tensor("kb1", [2 * N_TEXT, 1], f32, kind="ExternalInput").ap()
    kw2ts_h = nc.dram_tensor("kw2ts", [2 * N_TEXT, N_ATT], bf16, kind="ExternalInput").ap()
    kb2s_h = nc.dram_tensor("kb2s", [N_ATT, 1], f32, kind="ExternalInput").ap()

    out_attn_h = nc.dram_tensor("out_attn", [BPC, T1, T2], bf16, kind="ExternalOutput").ap()
    out_logp_h = nc.dram_tensor("out_logp", [BPC, T1, T2], bf16, kind="ExternalOutput").ap()

    with tile.TileContext(nc) as tc:
        from contextlib import ExitStack
        with ExitStack() as ctx:
            wpool = ctx.enter_context(tc.tile_pool(name="weights", bufs=1))
            iop = ctx.enter_context(tc.tile_pool(name="io", bufs=2))
            y1pool = ctx.enter_context(tc.tile_pool(name="y1", bufs=10))
            chunkp = ctx.enter_context(tc.tile_pool(name="chunk", bufs=4))
            smallp = ctx.enter_context(tc.tile_pool(name="small", bufs=8))
            ps_conv = ctx.enter_context(tc.tile_pool(name="ps_conv", bufs=3, space="PSUM"))
            ps_qk = ctx.enter_context(tc.tile_pool(name="ps_qk", bufs=3, space="PSUM"))
            ps_small = ctx.enter_context(tc.tile_pool(name="ps_small", bufs=2, space="PSUM"))

            # ---- constants & weights (loaded once) -----------------------
            ident_sb = wpool.tile([128, 128], bf16, name="ident_sb")
            nc.sync.dma_start(out=ident_sb[:], in_=ident_h[:, :])
            ones1 = wpool.tile([1, 128], bf16, name="ones1")
            nc.vector.memset(ones1[:], 1.0)
            ones80 = wpool.tile([N_ATT, 1], bf16, name="ones80")
            nc.vector.memset(ones80[:], 1.0)
            zeros500 = wpool.tile([128, QT], bf16, name="zeros500")
            nc.vector.memset(zeros500[:], 0.0)


            # ---- manually double-buffered persistent tiles ---------------
            # (zero halo cols / ones rows are initialized once, outside the
            # batch loop, and never rewritten)
            q_in2, q_fin2, b02, ks2 = [], [], [], []
            for i in range(2):
                q_in = wpool.tile([N_MEL, T1 + 2], bf16, name=f"q_in_{i}")
                nc.vector.memset(q_in[:, 0:1], 0.0)
                nc.vector.memset(q_in[:, T1 + 1:T1 + 2], 0.0)
                q_in2.append(q_in)
                q_fin = wpool.tile([97, T1], bf16, name=f"q_fin_{i}")
                nc.vector.memset(q_fin[64:96, :], 0.0)
                nc.vector.memset(q_fin[96:97, :], 1.0)
                q_fin2.append(q_fin)
                b0 = wpool.tile([97, T2], bf16, name=f"b0_{i}")
                nc.vector.memset(b0[64:96, :], 0.0)
                b02.append(b0)
                ks_c = []
                for c in range(4):
                    ks = wpool.tile([128, T2 + 2], bf16, name=f"ks_{c}_{i}")
                    nc.vector.memset(ks[:, 0:1], 0.0)
                    nc.vector.memset(ks[:, T2 + 1:T2 + 2], 0.0)
                    ks_c.append(ks)
                ks2.append(ks_c)

            # ---- input prefetch (one batch ahead) ------------------------
            def load_inputs(b):
                bb = b % 2
                st_all = iop.tile([100, 4 * N_TEXT], bf16, name="st_all", tag="st")
                nc.sync.dma_start(
                    out=st_all[:].rearrange("p (ti ch) -> p ti ch", ti=4),
                    in_=style_h[b].rearrange("(ti p) ch -> p ti ch", p=100))
                kt_all = iop.tile([128, 4 * T2], bf16, name="kt_all", tag="kt")
                nc.sync.dma_start(
                    out=kt_all[:].rearrange("p (c t) -> p c t", c=4),
                    in_=keys_h[b].rearrange("(c p) t -> p c t", p=128))
                mrow = smallp.tile([1, T2], bf16, name="mrow", tag="mrow")
                nc.sync.dma_start(out=mrow[0:1, :], in_=maskf_h[b:b + 1, :])
                nc.sync.dma_start(out=q_in2[bb][:, 1:T1 + 1], in_=queries_h[b, :, :])
                pr_all = iop.tile([128, 16 * T2], bf16, name="pr_all", tag="pr")
                nc.sync.dma_start(
                    out=pr_all[:, 0:NFULL * T2].rearrange("p (c t) -> p c t", c=NFULL),
                    in_=prior_h[b, 0:T1A, :].rearrange("(c p) t -> p c t", p=128))
                nc.sync.dma_start(
                    out=pr_all[0:W15, NFULL * T2:16 * T2],
                    in_=prior_h[b, T1A:T1, :])
                return st_all, kt_all, mrow, pr_all

            pending = load_inputs(0)

            kw1_sb = []
            for j in range(8):
                t = wpool.tile([128, 1536], bf16, name=f"kw1_{j}")
                nc.sync.dma_start(out=t[:], in_=kw1j_h[j])
                kw1_sb.append(t)
            qw1_sb = []
            for d in range(3):
                t = wpool.tile([N_MEL, 2 * N_MEL], bf16, name=f"qw1_{d}")
                nc.sync.dma_start(out=t[:], in_=qw1t_h[d, :, :])
                qw1_sb.append(t)
            qw2_a = wpool.tile([128, N_MEL], bf16, name="qw2_a")
            nc.sync.dma_start(out=qw2_a[:], in_=qw2t_h[0:128, :])
            qw2_b = wpool.tile([32, N_MEL], bf16, name="qw2_b")
            nc.sync.dma_start(out=qw2_b[:], in_=qw2t_h[128:160, :])
            qw3_sb = wpool.tile([N_MEL, N_ATT], bf16, name="qw3_sb")
            nc.sync.dma_start(out=qw3_sb[:], in_=qw3t_h[:, :])
            kw2_sb = []
            for c in range(8):
                t = wpool.tile([128, N_ATT], bf16, name=f"kw2_{c}")
                nc.sync.dma_start(out=t[:], in_=kw2ts_h[128 * c:128 * (c + 1), :])
                kw2_sb.append(t)

            qb1_a = wpool.tile([128, 1], f32, name="qb1_a")
            nc.sync.dma_start(out=qb1_a[:], in_=qb1_h[0:128, :])
            qb1_b = wpool.tile([32, 1], f32, name="qb1_b")
            nc.sync.dma_start(out=qb1_b[:], in_=qb1_h[128:160, :])
            qb2_sb = wpool.tile([N_MEL, 1], f32, name="qb2_sb")
            nc.sync.dma_start(out=qb2_sb[:], in_=qb2_h[:, :])
            qb3_sb = wpool.tile([N_ATT, 1], f32, name="qb3_sb")
            nc.sync.dma_start(out=qb3_sb[:], in_=qb3_h[:, :])
            kb1_sb = []
            for c in range(8):
                t = wpool.tile([128, 1], f32, name=f"kb1_{c}")
                nc.sync.dma_start(out=t[:], in_=kb1_h[128 * c:128 * (c + 1), :])
                kb1_sb.append(t)
            kb2s_sb = wpool.tile([N_ATT, 1], f32, name="kb2s_sb")
            nc.sync.dma_start(out=kb2s_sb[:], in_=kb2s_h[:, :])

            # ---- per-batch work ------------------------------------------
            for b in range(BPC):
                bb = b % 2
                q_in, q_fin, b0, ks_sb = q_in2[bb], q_fin2[bb], b02[bb], ks2[bb]
                st_all, kt_all, mrow, pr_all = pending
                if b + 1 < BPC:
                    pending = load_inputs(b + 1)

                for c in range(4):
                    ks_ps = ps_conv.tile([128, T2], bf16, name=f"ks_ps_{c}", tag="psc")
                    for ti in range(4):
                        nc.tensor.transpose(
                            ks_ps[:, 100 * ti:100 * (ti + 1)],
                            st_all[0:100, 512 * ti + 128 * c:512 * ti + 128 * (c + 1)],
                            ident_sb[0:100, 0:100],
                        )
                    nc.vector.tensor_add(ks_sb[c][:, 1:T2 + 1],
                                         kt_all[:, T2 * c:T2 * (c + 1)], ks_ps[:])

                # conv1 (k3, 512 -> 1024), bias+relu fused on Pool
                y1_sb = []
                for j in range(8):
                    c1 = ps_conv.tile([128, T2], f32, name=f"c1_{j}", tag="psc")
                    n = 0
                    for c in range(4):
                        for d in range(3):
                            nc.tensor.matmul(
                                c1[:],
                                kw1_sb[j][:, (d * 4 + c) * 128:(d * 4 + c + 1) * 128],
                                ks_sb[c][:, d:d + T2],
                                start=(n == 0), stop=(n == 11),
                            )
                            n += 1
                    y1 = y1pool.tile([128, T2], bf16, name=f"y1_{j}", tag="y1")
                    nc.vector.scalar_tensor_tensor(
                        y1[:], c1[:], kb1_sb[j][:], zeros500[:, 0:T2],
                        ALU.add, ALU.max)
                    y1_sb.append(y1)

                # conv2 (k1, 1024 -> 80), FOLD pre-applied; bias on DVE
                k_ps = ps_conv.tile([N_ATT, T2], f32, name="k_ps", tag="psc")
                for c in range(8):
                    nc.tensor.matmul(k_ps[:], kw2_sb[c][:], y1_sb[c][:],
                                     start=(c == 0), stop=(c == 7))
                nc.vector.tensor_scalar_add(b0[0:N_ATT, :], k_ps[:], kb2s_sb[:])

                # S*|k|^2 row into b0[80]:  sum(b0[0:80]^2) * K2SCALE
                ksq = chunkp.tile([N_ATT, T2], bf16, name="ksq", tag="ksq", bufs=2)
                nc.gpsimd.tensor_mul(ksq[:], b0[0:N_ATT, :], b0[0:N_ATT, :])
                k2_ps = ps_small.tile([1, T2], f32, name="k2_ps", tag="pss")
                nc.tensor.matmul(k2_ps[:], ones80[:], ksq[:], start=True, stop=True)
                nc.vector.tensor_scalar_mul(b0[96:97, :], k2_ps[:], K2SCALE)

                # mask row -> broadcast to [128, T2] (bf16), SBUF-only on Pool
                mmul = chunkp.tile([128, T2], bf16, name="mmul", tag="mmul", bufs=2)
                nc.gpsimd.partition_broadcast(mmul[:], mrow[0:1, :])

                # ---------- query path ----------
                for tc_i in range(4):
                    t0 = QT * tc_i
                    q1a = ps_conv.tile([128, QT], f32, name=f"q1a_{t0}", tag="psc")
                    for d in range(3):
                        nc.tensor.matmul(q1a[:], qw1_sb[d][:, 0:128],
                                         q_in[:, d + t0:d + t0 + QT],
                                         start=(d == 0), stop=(d == 2))
                    q1b = ps_conv.tile([32, QT], f32, name=f"q1b_{t0}", tag="psc")
                    for d in range(3):
                        nc.tensor.matmul(q1b[:], qw1_sb[d][:, 128:160],
                                         q_in[:, d + t0:d + t0 + QT],
                                         start=(d == 0), stop=(d == 2))
                    y1qa = chunkp.tile([128, QT], bf16, name=f"y1qa_{t0}", tag="y1qa", bufs=2)
                    nc.vector.scalar_tensor_tensor(
                        y1qa[:], q1a[:], qb1_a[:], zeros500[:],
                        ALU.add, ALU.max)
                    y1qb = chunkp.tile([32, QT], bf16, name=f"y1qb_{t0}", tag="y1qb", bufs=2)
                    nc.scalar.activation(y1qb[:], q1b[:], AF.Relu, bias=qb1_b[:])

                    q2 = ps_conv.tile([N_MEL, QT], f32, name=f"q2_{t0}", tag="psc")
                    nc.tensor.matmul(q2[:], qw2_a[:], y1qa[:], start=True, stop=False)
                    nc.tensor.matmul(q2[:], qw2_b[:], y1qb[:], start=False, stop=True)
                    q_mid = chunkp.tile([N_MEL, QT], bf16, name=f"q_mid_{t0}", tag="q_mid", bufs=2)
                    nc.vector.scalar_tensor_tensor(
                        q_mid[:], q2[:], qb2_sb[:], zeros500[0:N_MEL, :],
                        ALU.add, ALU.max)

                    q3 = ps_conv.tile([N_ATT, QT], f32, name=f"q3_{t0}", tag="psc")
                    nc.tensor.matmul(q3[:], qw3_sb[:], q_mid[:], start=True, stop=True)
                    nc.scalar.activation(q_fin[0:N_ATT, t0:t0 + QT], q3[:],
                                         AF.Identity, bias=qb3_sb[:])

                # ---------- attention chunks ----------
                u_all = iop.tile([128, 16 * T2], bf16, name="u_all", tag="uall")
                o1_all = iop.tile([128, 16 * T2], bf16, name="o1_all", tag="o1")
                o2_all = iop.tile([128, 16 * T2], bf16, name="o2_all", tag="o2")
                rrec_all = iop.tile([128, 16], f32, name="rrec_all", tag="rrec")

                # Last batch: two half-groups so its Ln/output-DMA tail
                # overlaps its own second-half Exp phase (costs 2 extra
                # activation-table loads, saves most of the drain tail).
                groups = [(0, 16)] if b < BPC - 1 else [(0, 8), (8, 16)]
                for (g0, g1) in groups:
                    # phase B: Exp only on Act
                    for ci in range(g0, g1):
                        w = 128 if ci < NFULL else W15
                        r0 = 128 * ci
                        cs = slice(T2 * ci, T2 * ci + T2)
                        p0 = ps_qk.tile([w, T2], f32, name=f"p0_{ci}", tag="psqk")
                        nc.tensor.matmul(p0[:], q_fin[:, r0:r0 + w], b0[:],
                                         start=True, stop=True)
                        e = chunkp.tile([w, T2], bf16, name=f"e_{ci}", tag="e", bufs=4)
                        rsum = smallp.tile([w, 1], f32, name=f"rsum_{ci}", tag="rsum")
                        nc.scalar.activation(e[:], p0[:], AF.Exp, accum_out=rsum[:])
                        nc.vector.reciprocal(rrec_all[0:w, ci:ci + 1], rsum[:])
                        # u'' = e * prior_eps  (shared by logp and attn paths)
                        nc.gpsimd.tensor_mul(u_all[0:w, cs], e[:], pr_all[0:w, cs])
                        # u = u'' * mask, masked row sum in the same pass
                        u = chunkp.tile([w, T2], bf16, name=f"u_{ci}", tag="u", bufs=4)
                        rsum2 = smallp.tile([w, 1], f32, name=f"rsum2_{ci}", tag="rsum2")
                        nc.vector.scalar_tensor_tensor(
                            u[:], u_all[0:w, cs], 1.0, mmul[0:w, :],
                            ALU.mult, ALU.mult, accum_out=rsum2[:])
                        rrec2 = smallp.tile([w, 1], f32, name=f"rrec2_{ci}", tag="rrec2")
                        nc.vector.reciprocal(rrec2[:], rsum2[:])
                        nc.vector.tensor_scalar_mul(o2_all[0:w, cs], u[:], rrec2[:])

                    if g1 == 16:
                        nc.sync.dma_start(
                            out=out_attn_h[b, 128 * g0:T1A, :].rearrange(
                                "(c p) t -> p c t", p=128),
                            in_=o2_all[:, g0 * T2:NFULL * T2].rearrange(
                                "p (c t) -> p c t", c=NFULL - g0))
                        nc.sync.dma_start(
                            out=out_attn_h[b, T1A:T1, :],
                            in_=o2_all[0:W15, NFULL * T2:16 * T2])
                    else:
                        nc.sync.dma_start(
                            out=out_attn_h[b, 128 * g0:128 * g1, :].rearrange(
                                "(c p) t -> p c t", p=128),
                            in_=o2_all[:, g0 * T2:g1 * T2].rearrange(
                                "p (c t) -> p c t", c=g1 - g0))

                    # phase A': Ln only on Act; logp = Ln(u'' * rrec).
                    # rrec_cp is a scheduling barrier: copying the group's
                    # rrec columns makes every Ln depend on all its chunks, so
                    # the list scheduler cannot interleave Lns with Exps (each
                    # interleave costs a 1283ns activation-table reload).
                    rrec_cp = smallp.tile([128, 16], f32, name=f"rrec_cp_{g0}",
                                          tag="rrec_cp", bufs=2)
                    nc.vector.tensor_copy(rrec_cp[:, g0:g1], rrec_all[:, g0:g1])
                    for ci in range(g0, g1):
                        w = 128 if ci < NFULL else W15
                        cs = slice(T2 * ci, T2 * ci + T2)
                        nc.scalar.activation(o1_all[0:w, cs], u_all[0:w, cs],
                                             AF.Ln, scale=rrec_cp[0:w, ci:ci + 1])
                        if ci == 7:
                            nc.scalar.dma_start(
                                out=out_logp_h[b, 0:1024, :].rearrange(
                                    "(c p) t -> p c t", p=128),
                                in_=o1_all[:, 0:8 * T2].rearrange(
                                    "p (c t) -> p c t", c=8))

                    if g1 == 16:
                        nc.scalar.dma_start(
                            out=out_logp_h[b, 1024:T1A, :].rearrange(
                                "(c p) t -> p c t", p=128),
                            in_=o1_all[:, 8 * T2:NFULL * T2].rearrange(
                                "p (c t) -> p c t", c=7))
                        nc.scalar.dma_start(
                            out=out_logp_h[b, T1A:T1, :],
                            in_=o1_all[0:W15, NFULL * T2:16 * T2])

    nc.compile()
    return nc


def get_program(mm_f32r=None):
    if "prog" not in _PROGRAM_CACHE:
        _PROGRAM_CACHE["prog"] = build_program()
    return _PROGRAM_CACHE["prog"]


def make_in_maps(inputs):
    """Host-side prep: shard per core, transpose/fold/cast weights."""
    bf = ml_dtypes.bfloat16
    queries = np.asarray(inputs["queries"], np.float32)
    keys = np.asarray(inputs["keys"], np.float32)
    mask = np.asarray(inputs["mask"])
    prior = np.asarray(inputs["attn_prior"], np.float32)
    style = np.asarray(inputs["style_emb"], np.float32)

    qw1 = np.asarray(inputs["qw1"], np.float32)
    qb1 = np.asarray(inputs["qb1"], np.float32)
    qw2 = np.asarray(inputs["qw2"], np.float32)
    qb2 = np.asarray(inputs["qb2"], np.float32)
    qw3 = np.asarray(inputs["qw3"], np.float32)
    qb3 = np.asarray(inputs["qb3"], np.float32)
    kw1 = np.asarray(inputs["kw1"], np.float32)
    kb1 = np.asarray(inputs["kb1"], np.float32)
    kw2 = np.asarray(inputs["kw2"], np.float32)
    kb2 = np.asarray(inputs["kb2"], np.float32)

    qw1t = np.ascontiguousarray(qw1.transpose(2, 1, 0)).astype(bf)   # [3, 80, 160]
    qw2t = np.ascontiguousarray(qw2[:, :, 0].T).astype(bf)           # [160, 80]
    qw3t = np.ascontiguousarray(qw3[:, :, 0].T).astype(bf)           # [80, 80]
    kw1t = np.ascontiguousarray(kw1.transpose(2, 1, 0))             # [3, 512, 1024]
    # [d, c, p, j, o] -> [j, p, d, c, o] -> [8, 128, 1536]
    kw1j = np.ascontiguousarray(
        kw1t.reshape(3, 4, 128, 8, 128).transpose(3, 2, 0, 1, 4)
    ).reshape(8, 128, 1536).astype(bf)
    kw2ts = np.ascontiguousarray(kw2[:, :, 0].T * FOLD).astype(bf)   # [1024, 80]
    kb2s = (kb2 * FOLD).reshape(-1, 1).astype(np.float32)

    maskf = np.where(mask[:, :, 0], 0.0, 1.0).astype(bf)             # [B, 400]
    ident = np.eye(128, dtype=np.float32).astype(bf)
    prior_eps = (prior + EPS).astype(bf)

    shared = dict(
        ident=ident,
        qw1t=qw1t, qb1=qb1.reshape(-1, 1).astype(np.float32),
        qw2t=qw2t, qb2=qb2.reshape(-1, 1).astype(np.float32),
        qw3t=qw3t, qb3=qb3.reshape(-1, 1).astype(np.float32),
        kw1j=kw1j, kb1=kb1.reshape(-1, 1).astype(np.float32),
        kw2ts=kw2ts, kb2s=kb2s,
    )
    in_maps = []
    for c in range(N_CORES):
        sl = slice(c * BPC, (c + 1) * BPC)
        m = dict(shared)
        m["queries"] = np.ascontiguousarray(queries[sl]).astype(bf)
        m["keys"] = np.ascontiguousarray(keys[sl]).astype(bf)
        m["style"] = np.ascontiguousarray(style[sl]).astype(bf)
        m["prior"] = np.ascontiguousarray(prior_eps[sl])
        m["maskf"] = np.ascontiguousarray(maskf[sl])
        in_maps.append(m)
    return in_maps


def kernel(**inputs):
    from concourse.bass_utils import run_bass_kernel_spmd

    nc = get_program()
    in_maps = make_in_maps(inputs)
    res = run_bass_kernel_spmd(nc, in_maps, list(range(N_CORES)))
    attn = np.concatenate(
        [np.asarray(r["out_attn"], np.float32) for r in res.results], axis=0)
    logp = np.concatenate(
        [np.asarray(r["out_logp"], np.float32) for r in res.results], axis=0)
    attn = attn.reshape(B, 1, T1, T2)
    logp = logp.reshape(B, 1, T1, T2)
    return attn, logp
